# revision 1
# baseline (speedup 1.0000x reference)
"""Two-layer GAT (PyG GATConv semantics) on 8 Trainium2 NeuronCores.

Sharding: edges partitioned by destination node (edge/dst partitioning).
Host sorts edges by dst; core c owns dst nodes [c*SHARD, (c+1)*SHARD) and all
their incoming edges.  Per 128-node tile, incoming edges are grouped into K
subtiles of 128.  Per subtile: indirect-DMA gather of the source rows
[h | a_src.h | a_dst.h] from a replicated (AllGather'ed) node table, one-hot
S^T built with is_equal vs an iota row, ad[dst] expanded with a PE transpose +
tiny matmul, attention weights ex = exp(leakyrelu(as_src + ad_dst)) (segment
max subtraction is skipped: logits are O(10), well within fp32 exp range, and
softmax is shift-invariant), features scaled by ex, and a PSUM-accumulated
matmul S^T.T @ [h*ex | ex] computes segment numerator+denominator together.
Graph mean-pool via one-hot(batch) matmul accumulated over tiles, AllReduce,
classifier + log_softmax on device.
"""

import sys

for _p in ("/opt/trn_rl_repo",):
    if _p not in sys.path:
        sys.path.insert(0, _p)

import numpy as np

P = 128
NEG_SLOPE = 0.2
EPS = 1e-16


def host_prep(inputs, cores=8):
    """Pure-host index/weight preprocessing. Returns (cfg, per_core_inputs)."""
    x = np.asarray(inputs["x"], dtype=np.float32)
    edge_index = np.asarray(inputs["edge_index"])
    batch = np.asarray(inputs["batch"])
    W1 = np.asarray(inputs["W1"], dtype=np.float32)
    a_src1 = np.asarray(inputs["a_src1"], dtype=np.float32)
    a_dst1 = np.asarray(inputs["a_dst1"], dtype=np.float32)
    b1 = np.asarray(inputs["b1"], dtype=np.float32)
    W2 = np.asarray(inputs["W2"], dtype=np.float32)
    a_src2 = np.asarray(inputs["a_src2"], dtype=np.float32)
    a_dst2 = np.asarray(inputs["a_dst2"], dtype=np.float32)
    b2 = np.asarray(inputs["b2"], dtype=np.float32)
    Wc = np.asarray(inputs["Wc"], dtype=np.float32)
    bc = np.asarray(inputs["bc"], dtype=np.float32)

    N, F = x.shape
    H1 = a_src1.shape[0]            # heads in layer 1 (2)
    HID = a_src1.shape[1]           # per-head hidden (64)
    D1 = H1 * HID                   # layer-1 output dim (128)
    NCLS = Wc.shape[1]
    G = 64

    NP = ((N + cores * P - 1) // (cores * P)) * (cores * P)
    SHARD = NP // cores
    NT = SHARD // P

    # --- weights: augmented projection matrices ---
    As1 = np.zeros((D1, H1), np.float32)
    Ad1 = np.zeros((D1, H1), np.float32)
    for h in range(H1):
        As1[h * HID:(h + 1) * HID, h] = a_src1[h]
        Ad1[h * HID:(h + 1) * HID, h] = a_dst1[h]
    # [F, D1 + 2*H1] : [W1 | W1@As | W1@Ad]
    W1aug = np.concatenate([W1, W1 @ As1, W1 @ Ad1], axis=1)
    R1 = W1aug.shape[1]             # 132

    W2aug = np.concatenate(
        [W2, W2 @ a_src2.T, W2 @ a_dst2.T, np.zeros((D1, 2), np.float32)],
        axis=1)                     # [128, 68]
    R2 = W2aug.shape[1]

    # --- edges sorted by dst, grouped per 128-node tile ---
    src = edge_index[0].astype(np.int64)
    dst = edge_index[1].astype(np.int64)
    order = np.argsort(dst, kind="stable")
    ss = src[order].astype(np.int32)
    ds = dst[order]
    tile_of = (ds // P).astype(np.int64)
    n_tiles = NP // P
    counts = np.bincount(tile_of, minlength=n_tiles)
    K = int(np.ceil(counts.max() / P))
    starts = np.cumsum(counts) - counts
    j = np.arange(len(ss)) - starts[tile_of]
    kk = (j // P).astype(np.int64)
    pp = (j % P).astype(np.int64)

    src_all = np.zeros((n_tiles, P, K), np.int32)      # pad: gather row 0
    rel_all = np.full((n_tiles, P, K), 255.0, np.float32)
    src_all[tile_of, pp, kk] = ss
    rel_all[tile_of, pp, kk] = (ds % P).astype(np.float32)

    bpad = np.full(NP, 255.0, np.float32)
    bpad[:N] = batch.astype(np.float32)

    xpad = np.zeros((NP, F), np.float32)
    xpad[:N] = x

    # --- constants (replicated) ---
    iota128 = np.tile(np.arange(P, dtype=np.float32), (P, 1))
    iota64 = np.tile(np.arange(G, dtype=np.float32), (P, 1))
    ident = np.eye(P, dtype=np.float32)
    b1rep = np.tile(b1, (P, 1))
    b2rep = np.tile(b2, (P, 1))
    bcrep = np.tile(bc, (G, 1))
    ones_col = np.ones((P, 1), np.float32)

    shared = {
        "W1aug": W1aug, "W2aug": W2aug, "iota128": iota128,
        "iota64": iota64, "ident": ident, "b1rep": b1rep, "b2rep": b2rep,
        "Wc": Wc, "bcrep": bcrep, "ones_col": ones_col,
    }
    per_core = []
    for c in range(cores):
        lo = c * SHARD
        per_core.append({
            **shared,
            "xT": np.ascontiguousarray(xpad[lo:lo + SHARD].T),
            "src_idx": np.ascontiguousarray(
                src_all[c * NT:(c + 1) * NT]),
            "dst_rel": np.ascontiguousarray(
                rel_all[c * NT:(c + 1) * NT]),
            "batchv": np.ascontiguousarray(
                bpad[lo:lo + SHARD].reshape(NT, P, 1)),
        })

    cfg = dict(N=N, F=F, H1=H1, HID=HID, D1=D1, NCLS=NCLS, G=G, NP=NP,
               SHARD=SHARD, NT=NT, K=K, R1=R1, R2=R2, cores=cores)
    return cfg, per_core


def build_program(cfg):
    import concourse.bacc as bacc
    import concourse.bass as bass
    import concourse.mybir as mybir
    import concourse.tile as tile

    f32 = mybir.dt.float32
    i32 = mybir.dt.int32
    AF = mybir.ActivationFunctionType
    OP = mybir.AluOpType

    F, H1, HID, D1 = cfg["F"], cfg["H1"], cfg["HID"], cfg["D1"]
    NCLS, G = cfg["NCLS"], cfg["G"]
    NP, SHARD, NT, K = cfg["NP"], cfg["SHARD"], cfg["NT"], cfg["K"]
    R1, R2, cores = cfg["R1"], cfg["R2"], cfg["cores"]

    nc = bacc.Bacc("TRN2", target_bir_lowering=False, debug=False)

    # inputs
    xT = nc.dram_tensor("xT", [F, SHARD], f32, kind="ExternalInput")
    src_idx = nc.dram_tensor("src_idx", [NT, P, K], i32, kind="ExternalInput")
    dst_rel = nc.dram_tensor("dst_rel", [NT, P, K], f32, kind="ExternalInput")
    batchv = nc.dram_tensor("batchv", [NT, P, 1], f32, kind="ExternalInput")
    W1aug = nc.dram_tensor("W1aug", [F, R1], f32, kind="ExternalInput")
    W2aug = nc.dram_tensor("W2aug", [D1, R2], f32, kind="ExternalInput")
    iota128 = nc.dram_tensor("iota128", [P, P], f32, kind="ExternalInput")
    iota64 = nc.dram_tensor("iota64", [P, G], f32, kind="ExternalInput")
    ident = nc.dram_tensor("ident", [P, P], f32, kind="ExternalInput")
    b1rep = nc.dram_tensor("b1rep", [P, D1], f32, kind="ExternalInput")
    b2rep = nc.dram_tensor("b2rep", [P, HID], f32, kind="ExternalInput")
    Wc = nc.dram_tensor("Wc", [HID, NCLS], f32, kind="ExternalInput")
    bcrep = nc.dram_tensor("bcrep", [G, NCLS], f32, kind="ExternalInput")
    ones_col = nc.dram_tensor("ones_col", [P, 1], f32, kind="ExternalInput")

    y = nc.dram_tensor("y", [G, NCLS], f32, kind="ExternalOutput")

    with tile.TileContext(nc) as tc:
        with (
            tc.tile_pool(name="const", bufs=1) as cpool,
            tc.tile_pool(name="work", bufs=4) as wpool,
            tc.tile_pool(name="small", bufs=3) as spool,
            tc.tile_pool(name="pacc", bufs=2, space="PSUM") as pacc,
            tc.tile_pool(name="ptr", bufs=2, space="PSUM") as ptr,
            tc.tile_pool(name="psm", bufs=2, space="PSUM") as psm,
            tc.tile_pool(name="ppool", bufs=1, space="PSUM") as ppool,
            tc.tile_pool(name="dram", bufs=1, space="DRAM") as dpool,
        ):
            # ---- load constants ----
            def cload(ap, shape, tag):
                t = cpool.tile(shape, f32, tag=tag)
                nc.sync.dma_start(out=t[:], in_=ap[:])
                return t

            w1_sb = cload(W1aug, [F, R1], "w1")
            w2_sb = cload(W2aug, [D1, R2], "w2")
            io128_sb = cload(iota128, [P, P], "io128")
            io64_sb = cload(iota64, [P, G], "io64")
            id_sb = cload(ident, [P, P], "id")
            b1_sb = cload(b1rep, [P, D1], "b1")
            b2_sb = cload(b2rep, [P, HID], "b2")
            wc_sb = cload(Wc, [HID, NCLS], "wc")
            bc_sb = cload(bcrep, [G, NCLS], "bc")
            ones_sb = cload(ones_col, [P, 1], "ones")

            # DRAM intermediates
            h1_shard = dpool.tile([SHARD, R1], f32, tag="h1s")
            h1_full = dpool.tile([NP, R1], f32, tag="h1f")
            h2_shard = dpool.tile([SHARD, R2], f32, tag="h2s")
            h2_full = dpool.tile([NP, R2], f32, tag="h2f")
            pool_in = dpool.tile([G, HID + 1], f32, tag="pin")
            pool_out = dpool.tile([G, HID + 1], f32, tag="pout")

            groups = [list(range(cores))]

            # ================= phase 0: h1_aug = x @ W1aug ================
            for t in range(NT):
                xt = wpool.tile([F, P], f32, tag="xt")
                nc.sync.dma_start(out=xt[:], in_=xT[:, t * P:(t + 1) * P])
                hT = ptr.tile([P, P], f32, tag="tr")
                nc.tensor.matmul(out=hT[:], lhsT=w1_sb[:, 0:D1], rhs=xt[:],
                                 start=True, stop=True)
                gT = psm.tile([4, P], f32, tag="sm")
                nc.tensor.matmul(out=gT[:], lhsT=w1_sb[:, D1:R1], rhs=xt[:],
                                 start=True, stop=True)
                hT_sb = wpool.tile([P, P], f32, tag="hT_sb")
                nc.vector.tensor_copy(out=hT_sb[:], in_=hT[:])
                gT_sb = spool.tile([4, P], f32, tag="gT_sb")
                nc.vector.tensor_copy(out=gT_sb[:], in_=gT[:])
                h_nm = ptr.tile([P, P], f32, tag="tr")
                nc.tensor.transpose(out=h_nm[:], in_=hT_sb[:], identity=id_sb[:])
                g_nm = psm.tile([P, 4], f32, tag="sm")
                nc.tensor.transpose(out=g_nm[:], in_=gT_sb[:], identity=id_sb[0:4, 0:4])
                haug = wpool.tile([P, R1], f32, tag="haug")
                nc.vector.tensor_copy(out=haug[:, 0:D1], in_=h_nm[:])
                nc.vector.tensor_copy(out=haug[:, D1:R1], in_=g_nm[:])
                nc.sync.dma_start(out=h1_shard[t * P:(t + 1) * P, :],
                                  in_=haug[:])

            nc.gpsimd.collective_compute(
                "AllGather", mybir.AluOpType.bypass,
                replica_groups=groups,
                ins=[h1_shard.opt()], outs=[h1_full.opt()])

            # ============== edge phase (shared for both layers) ============
            def edge_layer(table_full, table_shard, R, heads, FW, post):
                """R row elems; heads; FW per-head feature width.
                rhs cols = heads*FW features then `heads` ex columns.
                post(t, psum_acc) consumes the accumulated [P, ncol] psum."""
                ncol = heads * FW + heads
                aslo = heads * FW          # as columns in gathered row
                for t in range(NT):
                    si = spool.tile([P, K], i32, tag="si")
                    nc.sync.dma_start(out=si[:], in_=src_idx[t])
                    dr = spool.tile([P, K], f32, tag="dr")
                    nc.sync.dma_start(out=dr[:], in_=dst_rel[t])
                    adn = spool.tile([P, heads], f32, tag="adn")
                    nc.sync.dma_start(
                        out=adn[:],
                        in_=table_shard[t * P:(t + 1) * P,
                                        aslo + heads:aslo + 2 * heads])
                    acc = pacc.tile([P, ncol], f32, tag="acc")
                    for k in range(K):
                        g = wpool.tile([P, R], f32, tag="g")
                        nc.gpsimd.indirect_dma_start(
                            out=g[:], out_offset=None,
                            in_=table_full[:],
                            in_offset=bass.IndirectOffsetOnAxis(
                                ap=si[:, k:k + 1], axis=0))
                        st = wpool.tile([P, P], f32, tag="st")
                        nc.vector.tensor_tensor(
                            out=st[:], in0=dr[:, k:k + 1].to_broadcast([P, P]),
                            in1=io128_sb[:], op=OP.is_equal)
                        s_ps = ptr.tile([P, P], f32, tag="tr")
                        nc.tensor.transpose(out=s_ps[:], in_=st[:],
                                            identity=id_sb[:])
                        s_sb = wpool.tile([P, P], f32, tag="s_sb")
                        nc.vector.tensor_copy(out=s_sb[:], in_=s_ps[:])
                        adx = psm.tile([P, heads], f32, tag="sm")
                        nc.tensor.matmul(out=adx[:], lhsT=s_sb[:], rhs=adn[:],
                                         start=True, stop=True)
                        z = spool.tile([P, heads], f32, tag="z")
                        nc.vector.tensor_tensor(
                            out=z[:], in0=g[:, aslo:aslo + heads],
                            in1=adx[:], op=OP.add)
                        zl = spool.tile([P, heads], f32, tag="zl")
                        nc.vector.tensor_scalar_mul(out=zl[:], in0=z[:],
                                                    scalar1=NEG_SLOPE)
                        zm = spool.tile([P, heads], f32, tag="zm")
                        nc.vector.tensor_tensor(out=zm[:], in0=z[:],
                                                in1=zl[:], op=OP.max)
                        ex = spool.tile([P, heads], f32, tag="ex")
                        nc.scalar.activation(out=ex[:], in_=zm[:], func=AF.Exp)
                        for h in range(heads):
                            nc.vector.tensor_scalar_mul(
                                out=g[:, h * FW:(h + 1) * FW],
                                in0=g[:, h * FW:(h + 1) * FW],
                                scalar1=ex[:, h:h + 1])
                        nc.vector.tensor_copy(out=g[:, aslo:aslo + heads],
                                              in_=ex[:])
                        nc.tensor.matmul(out=acc[:], lhsT=st[:],
                                         rhs=g[:, 0:ncol],
                                         start=(k == 0), stop=(k == K - 1))
                    post(t, acc)

            # ---- layer 1 post: divide, +b1, ELU, project to h2_aug ----
            def post1(t, acc):
                heads, FW = H1, HID
                den = spool.tile([P, heads], f32, tag="den")
                nc.vector.tensor_scalar_add(
                    out=den[:], in0=acc[:, heads * FW:heads * FW + heads],
                    scalar1=EPS)
                rec = spool.tile([P, heads], f32, tag="rec")
                nc.vector.reciprocal(out=rec[:], in_=den[:])
                o = wpool.tile([P, D1], f32, tag="o")
                for h in range(heads):
                    nc.vector.tensor_scalar_mul(
                        out=o[:, h * FW:(h + 1) * FW],
                        in0=acc[:, h * FW:(h + 1) * FW],
                        scalar1=rec[:, h:h + 1])
                nc.vector.tensor_tensor(out=o[:], in0=o[:], in1=b1_sb[:],
                                        op=OP.add)
                # elu(x) = max(x, exp(min(x,0)) - 1)
                m0 = wpool.tile([P, D1], f32, tag="m0")
                nc.vector.tensor_scalar_min(out=m0[:], in0=o[:], scalar1=0.0)
                em = wpool.tile([P, D1], f32, tag="em")
                nc.scalar.activation(out=em[:], in_=m0[:], func=AF.Exp)
                nc.vector.tensor_scalar_add(out=em[:], in0=em[:], scalar1=-1.0)
                h2in = wpool.tile([P, D1], f32, tag="h2in")
                nc.vector.tensor_tensor(out=h2in[:], in0=o[:], in1=em[:],
                                        op=OP.max)
                # h2aug = h2in @ W2aug  (via transpose / matmul / transpose)
                hT2 = ptr.tile([P, P], f32, tag="tr")
                nc.tensor.transpose(out=hT2[:], in_=h2in[:], identity=id_sb[:])
                hT2_sb = wpool.tile([P, P], f32, tag="hT2_sb")
                nc.vector.tensor_copy(out=hT2_sb[:], in_=hT2[:])
                a2T = ptr.tile([R2, P], f32, tag="tr")
                nc.tensor.matmul(out=a2T[:], lhsT=w2_sb[:], rhs=hT2_sb[:],
                                 start=True, stop=True)
                a2T_sb = wpool.tile([R2, P], f32, tag="a2T_sb")
                nc.vector.tensor_copy(out=a2T_sb[:], in_=a2T[:])
                a2 = ptr.tile([P, R2], f32, tag="tr")
                nc.tensor.transpose(out=a2[:], in_=a2T_sb[:], identity=id_sb[0:R2, 0:R2])
                a2_sb = wpool.tile([P, R2], f32, tag="a2_sb")
                nc.vector.tensor_copy(out=a2_sb[:], in_=a2[:])
                nc.sync.dma_start(out=h2_shard[t * P:(t + 1) * P, :],
                                  in_=a2_sb[:])

            edge_layer(h1_full, h1_shard, R1, H1, HID, post1)

            nc.gpsimd.collective_compute(
                "AllGather", mybir.AluOpType.bypass,
                replica_groups=groups,
                ins=[h2_shard.opt()], outs=[h2_full.opt()])

            # ---- layer 2 post: divide, +b2, pool accumulate ----
            pool_ps = ppool.tile([G, HID + 1], f32, tag="pool_ps")

            def post2(t, acc):
                den = spool.tile([P, 1], f32, tag="den2")
                nc.vector.tensor_scalar_add(out=den[:], in0=acc[:, HID:HID + 1],
                                            scalar1=EPS)
                rec = spool.tile([P, 1], f32, tag="rec2")
                nc.vector.reciprocal(out=rec[:], in_=den[:])
                o = wpool.tile([P, HID], f32, tag="o2")
                nc.vector.tensor_scalar_mul(out=o[:], in0=acc[:, 0:HID],
                                            scalar1=rec[:, 0:1])
                nc.vector.tensor_tensor(out=o[:], in0=o[:], in1=b2_sb[:],
                                        op=OP.add)
                bv = spool.tile([P, 1], f32, tag="bv")
                nc.sync.dma_start(out=bv[:], in_=batchv[t])
                oh = wpool.tile([P, G], f32, tag="oh")
                nc.vector.tensor_tensor(
                    out=oh[:], in0=bv[:, 0:1].to_broadcast([P, G]),
                    in1=io64_sb[:], op=OP.is_equal)
                rp = wpool.tile([P, HID + 1], f32, tag="rp")
                nc.vector.tensor_copy(out=rp[:, 0:HID], in_=o[:])
                nc.vector.tensor_copy(out=rp[:, HID:HID + 1], in_=ones_sb[:])
                nc.tensor.matmul(out=pool_ps[:], lhsT=oh[:], rhs=rp[:],
                                 start=(t == 0), stop=(t == NT - 1))

            edge_layer(h2_full, h2_shard, R2, 1, HID, post2)

            # ================= pooling reduce + classifier ================
            pool_sb = spool.tile([G, HID + 1], f32, tag="pool_sb")
            nc.vector.tensor_copy(out=pool_sb[:], in_=pool_ps[:])
            nc.sync.dma_start(out=pool_in[:], in_=pool_sb[:])
            nc.gpsimd.collective_compute(
                "AllReduce", mybir.AluOpType.add,
                replica_groups=groups,
                ins=[pool_in.opt()], outs=[pool_out.opt()])
            pr = spool.tile([G, HID + 1], f32, tag="pr")
            nc.sync.dma_start(out=pr[:], in_=pool_out[:])
            c1 = spool.tile([G, 1], f32, tag="c1")
            nc.vector.tensor_scalar_max(out=c1[:], in0=pr[:, HID:HID + 1],
                                        scalar1=1.0)
            rc = spool.tile([G, 1], f32, tag="rc")
            nc.vector.reciprocal(out=rc[:], in_=c1[:])
            pooled = spool.tile([G, HID], f32, tag="pooled")
            nc.vector.tensor_scalar_mul(out=pooled[:], in0=pr[:, 0:HID],
                                        scalar1=rc[:, 0:1])
            pT = psm.tile([HID, G], f32, tag="sm")
            nc.tensor.transpose(out=pT[:], in_=pooled[:], identity=id_sb[0:G, 0:G])
            pT_sb = spool.tile([HID, G], f32, tag="pT_sb")
            nc.vector.tensor_copy(out=pT_sb[:], in_=pT[:])
            lgT = psm.tile([NCLS, G], f32, tag="sm")
            nc.tensor.matmul(out=lgT[:], lhsT=wc_sb[:], rhs=pT_sb[:],
                             start=True, stop=True)
            lgT_sb = spool.tile([NCLS, G], f32, tag="lgT_sb")
            nc.vector.tensor_copy(out=lgT_sb[:], in_=lgT[:])
            lg_ps = psm.tile([G, NCLS], f32, tag="sm")
            nc.tensor.transpose(out=lg_ps[:], in_=lgT_sb[:], identity=id_sb[0:NCLS, 0:NCLS])
            lg = spool.tile([G, NCLS], f32, tag="lg")
            nc.vector.tensor_tensor(out=lg[:], in0=lg_ps[:], in1=bc_sb[:],
                                    op=OP.add)
            mx = spool.tile([G, 1], f32, tag="mx")
            nc.vector.tensor_reduce(out=mx[:], in_=lg[:],
                                    axis=mybir.AxisListType.X, op=OP.max)
            tm = spool.tile([G, NCLS], f32, tag="tm")
            nc.vector.tensor_scalar(out=tm[:], in0=lg[:],
                                    scalar1=mx[:, 0:1], scalar2=None,
                                    op0=OP.subtract)
            e2 = spool.tile([G, NCLS], f32, tag="e2")
            nc.scalar.activation(out=e2[:], in_=tm[:], func=AF.Exp)
            sm = spool.tile([G, 1], f32, tag="sm")
            nc.vector.tensor_reduce(out=sm[:], in_=e2[:],
                                    axis=mybir.AxisListType.X, op=OP.add)
            ln = spool.tile([G, 1], f32, tag="ln")
            nc.scalar.activation(out=ln[:], in_=sm[:], func=AF.Ln)
            yt = spool.tile([G, NCLS], f32, tag="yt")
            nc.vector.tensor_scalar(out=yt[:], in0=tm[:],
                                    scalar1=ln[:, 0:1], scalar2=None,
                                    op0=OP.subtract)
            nc.sync.dma_start(out=y[:], in_=yt[:])

    nc.finalize()
    return nc


def kernel(**inputs) -> np.ndarray:
    from concourse import bass_utils

    cfg, per_core = host_prep(inputs, cores=8)
    nc = build_program(cfg)
    res = bass_utils.run_bass_kernel_spmd(
        nc, per_core, core_ids=list(range(cfg["cores"])))
    return np.asarray(res.results[0]["y"])


if __name__ == "__main__":
    import reference
    ins = reference.setup_inputs()
    out = kernel(**{k: np.asarray(v) for k, v in ins.items()})
    exp = np.asarray(reference.reference(**ins))
    err = np.abs(out - exp).max() / max(np.abs(exp).max(), 1e-12)
    print("Relative error:", err)



# revision 11
# speedup vs baseline: 1.0040x; 1.0040x over previous
"""Two-layer GAT (PyG GATConv semantics) on 8 Trainium2 NeuronCores.

Sharding: edges partitioned by destination node (edge/dst partitioning).
Host sorts edges by dst; core c owns dst nodes [c*SHARD, (c+1)*SHARD) and all
their incoming edges.  Per 128-node tile, incoming edges are grouped into K
subtiles of 128.  Per subtile: indirect-DMA gather of the source rows
[h | a_src.h | a_dst.h] from a replicated (AllGather'ed) node table, one-hot
S^T built with is_equal vs an iota row, ad[dst] expanded with a PE transpose +
tiny matmul, attention weights ex = exp(leakyrelu(as_src + ad_dst)) (segment
max subtraction is skipped: logits are O(10), well within fp32 exp range, and
softmax is shift-invariant), features scaled by ex, and a PSUM-accumulated
matmul S^T.T @ [h*ex | ex] computes segment numerator+denominator together.
Graph mean-pool via one-hot(batch) matmul accumulated over tiles, AllReduce,
classifier + log_softmax on device.
"""

import sys

for _p in ("/opt/trn_rl_repo",):
    if _p not in sys.path:
        sys.path.insert(0, _p)

import numpy as np

P = 128
NEG_SLOPE = 0.2
EPS = 1e-16


def host_prep(inputs, cores=8):
    """Pure-host index/weight preprocessing. Returns (cfg, per_core_inputs)."""
    x = np.asarray(inputs["x"], dtype=np.float32)
    edge_index = np.asarray(inputs["edge_index"])
    batch = np.asarray(inputs["batch"])
    W1 = np.asarray(inputs["W1"], dtype=np.float32)
    a_src1 = np.asarray(inputs["a_src1"], dtype=np.float32)
    a_dst1 = np.asarray(inputs["a_dst1"], dtype=np.float32)
    b1 = np.asarray(inputs["b1"], dtype=np.float32)
    W2 = np.asarray(inputs["W2"], dtype=np.float32)
    a_src2 = np.asarray(inputs["a_src2"], dtype=np.float32)
    a_dst2 = np.asarray(inputs["a_dst2"], dtype=np.float32)
    b2 = np.asarray(inputs["b2"], dtype=np.float32)
    Wc = np.asarray(inputs["Wc"], dtype=np.float32)
    bc = np.asarray(inputs["bc"], dtype=np.float32)

    N, F = x.shape
    H1 = a_src1.shape[0]            # heads in layer 1 (2)
    HID = a_src1.shape[1]           # per-head hidden (64)
    D1 = H1 * HID                   # layer-1 output dim (128)
    NCLS = Wc.shape[1]
    G = 64

    NP = ((N + cores * P - 1) // (cores * P)) * (cores * P)
    SHARD = NP // cores
    NT = SHARD // P

    # --- weights: augmented projection matrices ---
    As1 = np.zeros((D1, H1), np.float32)
    Ad1 = np.zeros((D1, H1), np.float32)
    for h in range(H1):
        As1[h * HID:(h + 1) * HID, h] = a_src1[h]
        Ad1[h * HID:(h + 1) * HID, h] = a_dst1[h]
    # [F, D1 + 2*H1] : [W1 | W1@As | W1@Ad]
    W1aug = np.concatenate([W1, W1 @ As1, W1 @ Ad1], axis=1)
    R1 = W1aug.shape[1]             # 132

    W2aug = np.concatenate(
        [W2, W2 @ a_src2.T, W2 @ a_dst2.T, np.zeros((D1, 2), np.float32)],
        axis=1)                     # [128, 68]
    R2 = W2aug.shape[1]

    # --- edges sorted by dst, grouped per 128-node tile ---
    src = edge_index[0].astype(np.int64)
    dst = edge_index[1].astype(np.int64)
    order = np.argsort(dst, kind="stable")
    ss = src[order].astype(np.int32)
    ds = dst[order]
    tile_of = (ds // P).astype(np.int64)
    n_tiles = NP // P
    counts = np.bincount(tile_of, minlength=n_tiles)
    K = int(np.ceil(counts.max() / P))
    starts = np.cumsum(counts) - counts
    j = np.arange(len(ss)) - starts[tile_of]
    kk = (j // P).astype(np.int64)
    pp = (j % P).astype(np.int64)

    src_all = np.zeros((n_tiles, P, K), np.int32)      # pad: gather row 0
    rel_all = np.full((n_tiles, P, K), 255.0, np.float32)
    src_all[tile_of, pp, kk] = ss
    rel_all[tile_of, pp, kk] = (ds % P).astype(np.float32)

    bpad = np.full(NP, 255.0, np.float32)
    bpad[:N] = batch.astype(np.float32)

    xpad = np.zeros((NP, F), np.float32)
    xpad[:N] = x

    # --- constants (replicated) ---
    iota128 = np.tile(np.arange(P, dtype=np.float32), (P, 1))
    iota64 = np.tile(np.arange(G, dtype=np.float32), (P, 1))
    ident = np.eye(P, dtype=np.float32)
    b1rep = np.tile(b1, (P, 1))
    b2rep = np.tile(b2, (P, 1))
    bcrep = np.tile(bc, (G, 1))
    ones_col = np.ones((P, 1), np.float32)

    shared = {
        "W1aug": W1aug, "W2aug": W2aug, "iota128": iota128,
        "iota64": iota64, "ident": ident, "b1rep": b1rep, "b2rep": b2rep,
        "Wc": Wc, "bcrep": bcrep, "ones_col": ones_col,
    }
    per_core = []
    for c in range(cores):
        lo = c * SHARD
        per_core.append({
            **shared,
            "xT": np.ascontiguousarray(xpad[lo:lo + SHARD].T),
            "src_idx": np.ascontiguousarray(
                src_all[c * NT:(c + 1) * NT]),
            "dst_rel": np.ascontiguousarray(
                rel_all[c * NT:(c + 1) * NT]),
            "batchv": np.ascontiguousarray(
                bpad[lo:lo + SHARD].reshape(NT, P, 1)),
        })

    cfg = dict(N=N, F=F, H1=H1, HID=HID, D1=D1, NCLS=NCLS, G=G, NP=NP,
               SHARD=SHARD, NT=NT, K=K, R1=R1, R2=R2, cores=cores)
    return cfg, per_core


def build_program(cfg):
    import concourse.bacc as bacc
    import concourse.bass as bass
    import concourse.mybir as mybir
    import concourse.tile as tile

    f32 = mybir.dt.float32
    i32 = mybir.dt.int32
    AF = mybir.ActivationFunctionType
    OP = mybir.AluOpType

    F, H1, HID, D1 = cfg["F"], cfg["H1"], cfg["HID"], cfg["D1"]
    NCLS, G = cfg["NCLS"], cfg["G"]
    NP, SHARD, NT, K = cfg["NP"], cfg["SHARD"], cfg["NT"], cfg["K"]
    R1, R2, cores = cfg["R1"], cfg["R2"], cfg["cores"]

    nc = bacc.Bacc("TRN2", target_bir_lowering=False, debug=False)

    # inputs
    xT = nc.dram_tensor("xT", [F, SHARD], f32, kind="ExternalInput")
    src_idx = nc.dram_tensor("src_idx", [NT, P, K], i32, kind="ExternalInput")
    dst_rel = nc.dram_tensor("dst_rel", [NT, P, K], f32, kind="ExternalInput")
    batchv = nc.dram_tensor("batchv", [NT, P, 1], f32, kind="ExternalInput")
    W1aug = nc.dram_tensor("W1aug", [F, R1], f32, kind="ExternalInput")
    W2aug = nc.dram_tensor("W2aug", [D1, R2], f32, kind="ExternalInput")
    iota128 = nc.dram_tensor("iota128", [P, P], f32, kind="ExternalInput")
    iota64 = nc.dram_tensor("iota64", [P, G], f32, kind="ExternalInput")
    ident = nc.dram_tensor("ident", [P, P], f32, kind="ExternalInput")
    b1rep = nc.dram_tensor("b1rep", [P, D1], f32, kind="ExternalInput")
    b2rep = nc.dram_tensor("b2rep", [P, HID], f32, kind="ExternalInput")
    Wc = nc.dram_tensor("Wc", [HID, NCLS], f32, kind="ExternalInput")
    bcrep = nc.dram_tensor("bcrep", [G, NCLS], f32, kind="ExternalInput")
    ones_col = nc.dram_tensor("ones_col", [P, 1], f32, kind="ExternalInput")

    y = nc.dram_tensor("y", [G, NCLS], f32, kind="ExternalOutput")

    with tile.TileContext(nc) as tc:
        with (
            tc.tile_pool(name="const", bufs=1) as cpool,
            tc.tile_pool(name="work", bufs=4) as wpool,
            tc.tile_pool(name="small", bufs=3) as spool,
            tc.tile_pool(name="pacc", bufs=2, space="PSUM") as pacc,
            tc.tile_pool(name="ptr", bufs=2, space="PSUM") as ptr,
            tc.tile_pool(name="psm", bufs=2, space="PSUM") as psm,
            tc.tile_pool(name="ppool", bufs=1, space="PSUM") as ppool,
            tc.tile_pool(name="dram", bufs=1, space="DRAM") as dpool,
        ):
            # ---- load constants ----
            def cload(ap, shape, tag):
                t = cpool.tile(shape, f32, tag=tag)
                nc.sync.dma_start(out=t[:], in_=ap[:])
                return t

            w1_sb = cload(W1aug, [F, R1], "w1")
            w2_sb = cload(W2aug, [D1, R2], "w2")
            io128_sb = cload(iota128, [P, P], "io128")
            io64_sb = cload(iota64, [P, G], "io64")
            id_sb = cload(ident, [P, P], "id")
            b1_sb = cload(b1rep, [P, D1], "b1")
            b2_sb = cload(b2rep, [P, HID], "b2")
            wc_sb = cload(Wc, [HID, NCLS], "wc")
            bc_sb = cload(bcrep, [G, NCLS], "bc")
            ones_sb = cload(ones_col, [P, 1], "ones")

            # DRAM intermediates
            h1_shard = dpool.tile([SHARD, R1], f32, tag="h1s")
            h1_full = dpool.tile([NP, R1], f32, tag="h1f")
            h2_shard = dpool.tile([SHARD, R2], f32, tag="h2s")
            h2_full = dpool.tile([NP, R2], f32, tag="h2f")
            pool_in = dpool.tile([G, HID + 1], f32, tag="pin")
            pool_out = dpool.tile([G, HID + 1], f32, tag="pout")

            groups = [list(range(cores))]

            # ================= phase 0: h1_aug = x @ W1aug ================
            for t in range(NT):
                xt = wpool.tile([F, P], f32, tag="xt")
                nc.sync.dma_start(out=xt[:], in_=xT[:, t * P:(t + 1) * P])
                hT = ptr.tile([P, P], f32, tag="tr")
                nc.tensor.matmul(out=hT[:], lhsT=w1_sb[:, 0:D1], rhs=xt[:],
                                 start=True, stop=True)
                gT = psm.tile([4, P], f32, tag="sm")
                nc.tensor.matmul(out=gT[:], lhsT=w1_sb[:, D1:R1], rhs=xt[:],
                                 start=True, stop=True)
                hT_sb = wpool.tile([P, P], f32, tag="hT_sb")
                nc.vector.tensor_copy(out=hT_sb[:], in_=hT[:])
                gT_sb = spool.tile([4, P], f32, tag="gT_sb")
                nc.vector.tensor_copy(out=gT_sb[:], in_=gT[:])
                h_nm = ptr.tile([P, P], f32, tag="tr")
                nc.tensor.transpose(out=h_nm[:], in_=hT_sb[:], identity=id_sb[:])
                g_nm = psm.tile([P, 4], f32, tag="sm")
                nc.tensor.transpose(out=g_nm[:], in_=gT_sb[:], identity=id_sb[0:4, 0:4])
                haug = wpool.tile([P, R1], f32, tag="haug")
                nc.vector.tensor_copy(out=haug[:, 0:D1], in_=h_nm[:])
                nc.vector.tensor_copy(out=haug[:, D1:R1], in_=g_nm[:])
                nc.sync.dma_start(out=h1_shard[t * P:(t + 1) * P, :],
                                  in_=haug[:])

            nc.gpsimd.collective_compute(
                "AllGather", mybir.AluOpType.bypass,
                replica_groups=groups,
                ins=[h1_shard.opt()], outs=[h1_full.opt()])

            # ============== edge phase (shared for both layers) ============
            def edge_layer(table_full, table_shard, R, heads, FW, post):
                """R row elems; heads; FW per-head feature width.
                rhs cols = heads*FW features then `heads` ex columns.
                post(t, psum_acc) consumes the accumulated [P, ncol] psum."""
                ncol = heads * FW + heads
                aslo = heads * FW          # as columns in gathered row
                for t in range(NT):
                    si = spool.tile([P, K], i32, tag="si")
                    nc.sync.dma_start(out=si[:], in_=src_idx[t])
                    dr = spool.tile([P, K], f32, tag="dr")
                    nc.sync.dma_start(out=dr[:], in_=dst_rel[t])
                    adn = spool.tile([P, heads], f32, tag="adn")
                    nc.sync.dma_start(
                        out=adn[:],
                        in_=table_shard[t * P:(t + 1) * P,
                                        aslo + heads:aslo + 2 * heads])
                    acc = pacc.tile([P, ncol], f32, tag="acc")
                    for k in range(K):
                        g = wpool.tile([P, R], f32, tag="g")
                        nc.gpsimd.indirect_dma_start(
                            out=g[:], out_offset=None,
                            in_=table_full[:],
                            in_offset=bass.IndirectOffsetOnAxis(
                                ap=si[:, k:k + 1], axis=0))
                        st = wpool.tile([P, P], f32, tag="st")
                        nc.vector.tensor_tensor(
                            out=st[:], in0=dr[:, k:k + 1].to_broadcast([P, P]),
                            in1=io128_sb[:], op=OP.is_equal)
                        s_ps = ptr.tile([P, P], f32, tag="tr")
                        nc.tensor.transpose(out=s_ps[:], in_=st[:],
                                            identity=id_sb[:])
                        s_sb = wpool.tile([P, P], f32, tag="s_sb")
                        nc.vector.tensor_copy(out=s_sb[:], in_=s_ps[:])
                        adx = psm.tile([P, heads], f32, tag="sm")
                        nc.tensor.matmul(out=adx[:], lhsT=s_sb[:], rhs=adn[:],
                                         start=True, stop=True)
                        z = spool.tile([P, heads], f32, tag="z")
                        nc.vector.tensor_tensor(
                            out=z[:], in0=g[:, aslo:aslo + heads],
                            in1=adx[:], op=OP.add)
                        zl = spool.tile([P, heads], f32, tag="zl")
                        nc.vector.tensor_scalar_mul(out=zl[:], in0=z[:],
                                                    scalar1=NEG_SLOPE)
                        zm = spool.tile([P, heads], f32, tag="zm")
                        nc.vector.tensor_tensor(out=zm[:], in0=z[:],
                                                in1=zl[:], op=OP.max)
                        ex = spool.tile([P, heads], f32, tag="ex")
                        nc.scalar.activation(out=ex[:], in_=zm[:], func=AF.Exp)
                        for h in range(heads):
                            nc.vector.tensor_scalar_mul(
                                out=g[:, h * FW:(h + 1) * FW],
                                in0=g[:, h * FW:(h + 1) * FW],
                                scalar1=ex[:, h:h + 1])
                        nc.vector.tensor_copy(out=g[:, aslo:aslo + heads],
                                              in_=ex[:])
                        nc.tensor.matmul(out=acc[:], lhsT=st[:],
                                         rhs=g[:, 0:ncol],
                                         start=(k == 0), stop=(k == K - 1))
                    post(t, acc)

            # ---- layer 1 post: divide, +b1, ELU, project to h2_aug ----
            def post1(t, acc):
                heads, FW = H1, HID
                den = spool.tile([P, heads], f32, tag="den")
                nc.vector.tensor_scalar_add(
                    out=den[:], in0=acc[:, heads * FW:heads * FW + heads],
                    scalar1=EPS)
                rec = spool.tile([P, heads], f32, tag="rec")
                nc.vector.reciprocal(out=rec[:], in_=den[:])
                o = wpool.tile([P, D1], f32, tag="o")
                for h in range(heads):
                    nc.vector.tensor_scalar_mul(
                        out=o[:, h * FW:(h + 1) * FW],
                        in0=acc[:, h * FW:(h + 1) * FW],
                        scalar1=rec[:, h:h + 1])
                nc.vector.tensor_tensor(out=o[:], in0=o[:], in1=b1_sb[:],
                                        op=OP.add)
                # elu(x) = max(x, exp(min(x,0)) - 1)
                m0 = wpool.tile([P, D1], f32, tag="m0")
                nc.vector.tensor_scalar_min(out=m0[:], in0=o[:], scalar1=0.0)
                em = wpool.tile([P, D1], f32, tag="em")
                nc.scalar.activation(out=em[:], in_=m0[:], func=AF.Exp)
                nc.vector.tensor_scalar_add(out=em[:], in0=em[:], scalar1=-1.0)
                h2in = wpool.tile([P, D1], f32, tag="h2in")
                nc.vector.tensor_tensor(out=h2in[:], in0=o[:], in1=em[:],
                                        op=OP.max)
                # h2aug = h2in @ W2aug  (via transpose / matmul / transpose)
                hT2 = ptr.tile([P, P], f32, tag="tr")
                nc.tensor.transpose(out=hT2[:], in_=h2in[:], identity=id_sb[:])
                hT2_sb = wpool.tile([P, P], f32, tag="hT2_sb")
                nc.vector.tensor_copy(out=hT2_sb[:], in_=hT2[:])
                a2T = ptr.tile([R2, P], f32, tag="tr")
                nc.tensor.matmul(out=a2T[:], lhsT=w2_sb[:], rhs=hT2_sb[:],
                                 start=True, stop=True)
                a2T_sb = wpool.tile([R2, P], f32, tag="a2T_sb")
                nc.vector.tensor_copy(out=a2T_sb[:], in_=a2T[:])
                a2 = ptr.tile([P, R2], f32, tag="tr")
                nc.tensor.transpose(out=a2[:], in_=a2T_sb[:], identity=id_sb[0:R2, 0:R2])
                a2_sb = wpool.tile([P, R2], f32, tag="a2_sb")
                nc.vector.tensor_copy(out=a2_sb[:], in_=a2[:])
                nc.sync.dma_start(out=h2_shard[t * P:(t + 1) * P, :],
                                  in_=a2_sb[:])

            edge_layer(h1_full, h1_shard, R1, H1, HID, post1)

            nc.gpsimd.collective_compute(
                "AllGather", mybir.AluOpType.bypass,
                replica_groups=groups,
                ins=[h2_shard.opt()], outs=[h2_full.opt()])

            # ---- layer 2 post: divide, +b2, pool accumulate ----
            pool_ps = ppool.tile([G, HID + 1], f32, tag="pool_ps")

            def post2(t, acc):
                den = spool.tile([P, 1], f32, tag="den2")
                nc.vector.tensor_scalar_add(out=den[:], in0=acc[:, HID:HID + 1],
                                            scalar1=EPS)
                rec = spool.tile([P, 1], f32, tag="rec2")
                nc.vector.reciprocal(out=rec[:], in_=den[:])
                o = wpool.tile([P, HID], f32, tag="o2")
                nc.vector.tensor_scalar_mul(out=o[:], in0=acc[:, 0:HID],
                                            scalar1=rec[:, 0:1])
                nc.vector.tensor_tensor(out=o[:], in0=o[:], in1=b2_sb[:],
                                        op=OP.add)
                bv = spool.tile([P, 1], f32, tag="bv")
                nc.sync.dma_start(out=bv[:], in_=batchv[t])
                oh = wpool.tile([P, G], f32, tag="oh")
                nc.vector.tensor_tensor(
                    out=oh[:], in0=bv[:, 0:1].to_broadcast([P, G]),
                    in1=io64_sb[:], op=OP.is_equal)
                rp = wpool.tile([P, HID + 1], f32, tag="rp")
                nc.vector.tensor_copy(out=rp[:, 0:HID], in_=o[:])
                nc.vector.tensor_copy(out=rp[:, HID:HID + 1], in_=ones_sb[:])
                nc.tensor.matmul(out=pool_ps[:], lhsT=oh[:], rhs=rp[:],
                                 start=(t == 0), stop=(t == NT - 1))

            edge_layer(h2_full, h2_shard, R2, 1, HID, post2)

            # ================= pooling reduce + classifier ================
            pool_sb = spool.tile([G, HID + 1], f32, tag="pool_sb")
            nc.vector.tensor_copy(out=pool_sb[:], in_=pool_ps[:])
            nc.sync.dma_start(out=pool_in[:], in_=pool_sb[:])
            nc.gpsimd.collective_compute(
                "AllReduce", mybir.AluOpType.add,
                replica_groups=groups,
                ins=[pool_in.opt()], outs=[pool_out.opt()])
            pr = spool.tile([G, HID + 1], f32, tag="pr")
            nc.sync.dma_start(out=pr[:], in_=pool_out[:])
            c1 = spool.tile([G, 1], f32, tag="c1")
            nc.vector.tensor_scalar_max(out=c1[:], in0=pr[:, HID:HID + 1],
                                        scalar1=1.0)
            rc = spool.tile([G, 1], f32, tag="rc")
            nc.vector.reciprocal(out=rc[:], in_=c1[:])
            pooled = spool.tile([G, HID], f32, tag="pooled")
            nc.vector.tensor_scalar_mul(out=pooled[:], in0=pr[:, 0:HID],
                                        scalar1=rc[:, 0:1])
            pT = psm.tile([HID, G], f32, tag="sm")
            nc.tensor.transpose(out=pT[:], in_=pooled[:], identity=id_sb[0:G, 0:G])
            pT_sb = spool.tile([HID, G], f32, tag="pT_sb")
            nc.vector.tensor_copy(out=pT_sb[:], in_=pT[:])
            lgT = psm.tile([NCLS, G], f32, tag="sm")
            nc.tensor.matmul(out=lgT[:], lhsT=wc_sb[:], rhs=pT_sb[:],
                             start=True, stop=True)
            lgT_sb = spool.tile([NCLS, G], f32, tag="lgT_sb")
            nc.vector.tensor_copy(out=lgT_sb[:], in_=lgT[:])
            lg_ps = psm.tile([G, NCLS], f32, tag="sm")
            nc.tensor.transpose(out=lg_ps[:], in_=lgT_sb[:], identity=id_sb[0:NCLS, 0:NCLS])
            lg = spool.tile([G, NCLS], f32, tag="lg")
            nc.vector.tensor_tensor(out=lg[:], in0=lg_ps[:], in1=bc_sb[:],
                                    op=OP.add)
            mx = spool.tile([G, 1], f32, tag="mx")
            nc.vector.tensor_reduce(out=mx[:], in_=lg[:],
                                    axis=mybir.AxisListType.X, op=OP.max)
            tm = spool.tile([G, NCLS], f32, tag="tm")
            nc.vector.tensor_scalar(out=tm[:], in0=lg[:],
                                    scalar1=mx[:, 0:1], scalar2=None,
                                    op0=OP.subtract)
            e2 = spool.tile([G, NCLS], f32, tag="e2")
            nc.scalar.activation(out=e2[:], in_=tm[:], func=AF.Exp)
            sm = spool.tile([G, 1], f32, tag="sm")
            nc.vector.tensor_reduce(out=sm[:], in_=e2[:],
                                    axis=mybir.AxisListType.X, op=OP.add)
            ln = spool.tile([G, 1], f32, tag="ln")
            nc.scalar.activation(out=ln[:], in_=sm[:], func=AF.Ln)
            yt = spool.tile([G, NCLS], f32, tag="yt")
            nc.vector.tensor_scalar(out=yt[:], in0=tm[:],
                                    scalar1=ln[:, 0:1], scalar2=None,
                                    op0=OP.subtract)
            nc.sync.dma_start(out=y[:], in_=yt[:])

    nc.finalize()
    return nc


def kernel(**inputs) -> np.ndarray:
    from concourse import bass_utils

    cfg, per_core = host_prep(inputs, cores=8)
    nc = build_program(cfg)
    res = bass_utils.run_bass_kernel_spmd(
        nc, per_core, core_ids=list(range(cfg["cores"])))
    return np.asarray(res.results[0]["y"])


if __name__ == "__main__":
    import reference
    ins = reference.setup_inputs()
    out = kernel(**{k: np.asarray(v) for k, v in ins.items()})
    exp = np.asarray(reference.reference(**ins))
    err = np.abs(out - exp).max() / max(np.abs(exp).max(), 1e-12)
    print("Relative error:", err)

